# revision 1
# baseline (speedup 1.0000x reference)
"""Trainium2 Bass kernel for music-transformer relative attention.

Shapes (hardcoded): x [2, 2048, 1024], 16 heads x 64 dims, MAXLEN == N == 2048.
Sharding: 8 cores = 2 batches x 4 head-groups (4 heads each). Each core computes
its heads' attention and a partial output projection; host sums the 4 partials
per batch and adds the bias.

Per-core kernel (all matmuls bf16, fp32 PSUM accumulation, fp32 softmax):
  QT/KT [64*4, N] transposed layout, V [N, 64*4] natural layout.
  srel: P[r, j] = q_{r-1} . e_{m0+j} computed per 128-row block, written
  contiguously to a DRAM scratch, re-read with row stride (W-1) -> the
  music-transformer skew becomes a plain strided DMA. Diagonal zero and the
  causal -1e9 mask are baked into columns of P before the bounce.
  Scores: PSUM = QK + I.T @ S (identity matmul accumulates srel+mask into the
  same PSUM bank), exp on ACT (scale=1/8, no max-subtraction needed), A^T via
  PE transpose, A^T @ [V | 1] accumulates out^T and row-sums, normalize via a
  reciprocal broadcast matmul, then out-proj from the transposed layout.
"""

import sys

sys.path.insert(0, "/opt/trn_rl_repo")

import numpy as np
import ml_dtypes

import concourse.bass as bass
import concourse.tile as tile
from concourse import bacc
from concourse import mybir
from concourse.bass_utils import run_bass_kernel_spmd
from concourse.masks import make_identity

BF = mybir.dt.bfloat16
F32 = mybir.dt.float32
N = 2048
D = 1024
HD = 64
HPC = 4          # heads per core
DC = HPC * HD    # 256 head dims per core
NB = N // 128    # 16 row blocks
WMAX = N + 127   # max scratch row width

_CACHE = {}


def _build_nc(repeat=1):
    nc = bacc.Bacc()
    xT = nc.dram_tensor("xT", [D, N], BF, kind="ExternalInput")
    wqT = nc.dram_tensor("wqT", [D, DC], BF, kind="ExternalInput")
    wkT = nc.dram_tensor("wkT", [D, DC], BF, kind="ExternalInput")
    wvT = nc.dram_tensor("wvT", [D, DC], BF, kind="ExternalInput")
    eT = nc.dram_tensor("eT", [DC, N], BF, kind="ExternalInput")
    wpT = nc.dram_tensor("wpT", [DC, D], BF, kind="ExternalInput")
    outp = nc.dram_tensor("outp", [N, D], F32, kind="ExternalOutput")
    scratch = nc.dram_tensor("scratch", [HPC * NB * 128 * WMAX], BF)

    from contextlib import ExitStack

    with tile.TileContext(nc) as tc, ExitStack() as ctx:
        pers = ctx.enter_context(tc.tile_pool(name="pers", bufs=1))
        psA = ctx.enter_context(tc.tile_pool(name="psA", bufs=4, space="PSUM"))
        psB = ctx.enter_context(tc.tile_pool(name="psB", bufs=4, space="PSUM"))
        pp = ctx.enter_context(tc.tile_pool(name="pp", bufs=4))
        ss = ctx.enter_context(tc.tile_pool(name="ss", bufs=6))
        aa = ctx.enter_context(tc.tile_pool(name="aa", bufs=6))
        atp = ctx.enter_context(tc.tile_pool(name="atp", bufs=4))
        oo = ctx.enter_context(tc.tile_pool(name="oo", bufs=4))
        llp = ctx.enter_context(tc.tile_pool(name="llp", bufs=3))
        if True:
            # ---- persistent SBUF tensors ----
            xt = [pers.tile([128, N], BF, tag=f"xt{i}", name=f"xt{i}") for i in range(8)]
            wq = [pers.tile([128, DC], BF, tag=f"wq{i}", name=f"wq{i}") for i in range(8)]
            wk = [pers.tile([128, DC], BF, tag=f"wk{i}", name=f"wk{i}") for i in range(8)]
            wv = [pers.tile([128, DC], BF, tag=f"wv{i}", name=f"wv{i}") for i in range(8)]
            wp = [pers.tile([128, D], BF, tag=f"wp{i}", name=f"wp{i}") for i in range(2)]
            et = [pers.tile([128, N], BF, tag=f"et{i}", name=f"et{i}") for i in range(2)]
            qt = [pers.tile([128, N + 1], BF, tag=f"qt{i}", name=f"qt{i}") for i in range(2)]
            kt = [pers.tile([128, N], BF, tag=f"kt{i}", name=f"kt{i}") for i in range(2)]
            vaug = [pers.tile([128, HPC * (HD + 1)], BF, tag=f"va{i}", name=f"va{i}") for i in range(NB)]
            aot = [pers.tile([128, N], BF, tag=f"ao{i}", name=f"ao{i}") for i in range(2)]
            ident = pers.tile([128, 128], BF, tag="ident", name="ident")
            ones = pers.tile([1, 64], F32, tag="ones", name="ones")

            make_identity(nc, ident[:])
            nc.gpsimd.memset(ones[:], 1.0)
            for g in range(2):
                nc.gpsimd.memset(qt[g][:, 0:1], 0.0)

            for i in range(8):
                nc.sync.dma_start(xt[i][:], xT[bass.ts(i, 128), :])
                nc.sync.dma_start(wq[i][:], wqT[bass.ts(i, 128), :])
                nc.sync.dma_start(wk[i][:], wkT[bass.ts(i, 128), :])
                nc.sync.dma_start(wv[i][:], wvT[bass.ts(i, 128), :])
            for g in range(2):
                nc.sync.dma_start(wp[g][:], wpT[bass.ts(g, 128), :])
                nc.sync.dma_start(et[g][:], eT[bass.ts(g, 128), :])

            # ---- projections: QT/KT transposed layout ----
            for _rep in range(repeat):
              for g in range(2):
                  for nchunk in range(4):
                      ps = psA.tile([128, 512], F32, tag="mm", name="mm")
                      for kc in range(8):
                          nc.tensor.matmul(
                              ps[:], wq[kc][:, bass.ts(g, 128)],
                              xt[kc][:, bass.ts(nchunk, 512)],
                              start=(kc == 0), stop=(kc == 7))
                      nc.scalar.copy(qt[g][:, 1 + nchunk * 512:1 + (nchunk + 1) * 512], ps[:])
                      ps2 = psA.tile([128, 512], F32, tag="mm", name="mm")
                      for kc in range(8):
                          nc.tensor.matmul(
                              ps2[:], wk[kc][:, bass.ts(g, 128)],
                              xt[kc][:, bass.ts(nchunk, 512)],
                              start=(kc == 0), stop=(kc == 7))
                      nc.scalar.copy(kt[g][:, bass.ts(nchunk, 512)], ps2[:])

              # ---- V natural layout + ones column ----
              for i in range(NB):
                  ps = psA.tile([128, DC], F32, tag="mm", name="mm")
                  for kc in range(8):
                      nc.tensor.matmul(
                          ps[:], xt[kc][:, bass.ts(i, 128)], wv[kc][:],
                          start=(kc == 0), stop=(kc == 7))
                  nc.gpsimd.memset(vaug[i][:], 1.0)
                  for h in range(HPC):
                      nc.vector.tensor_copy(
                          vaug[i][:, h * 65:h * 65 + 64], ps[:, bass.ts(h, 64)])

              # Funnel cross-engine deps into PE's observed clock so no real
              # matmul needs >2 sync waits (walrus MM wait-slot limit): dummy
              # [1,1] matmuls reading each phase-boundary tensor, cycling both
              # PSUM pools to also absorb their slot-release sems.
              srcs = [et[0], et[1], qt[0], qt[1], kt[0], kt[1], wp[0], wp[1]]
              for i, src in enumerate(srcs):
                  if i % 2 == 0:
                      ps_d = psA.tile([1, 1], F32, tag="mm", name="mm")
                  else:
                      ps_d = psB.tile([1, 1], F32, tag="sm", name="sm")
                  nc.tensor.matmul(ps_d[0:1, 0:1], src[0:1, 1:2], src[0:1, 1:2],
                                   start=True, stop=True)

              # ---- P bands (srel) + skew bounce writes ----
              for r0i in range(NB):
                  for h in range(HPC):
                      g, ho = h // 2, 64 * (h % 2)
                      c_max = 128 * r0i + 128
                      valid = c_max - 1          # data cols [0, valid)
                      W = c_max + 127            # scratch row width
                      m0 = N - valid             # first embedding index
                      p_sb = pp.tile([128, WMAX], BF, tag="psb", name="psb")
                      for c0 in range(0, valid, 512):
                          w = min(512, valid - c0)
                          ps = psA.tile([128, 512], F32, tag="mm", name="mm")
                          nc.tensor.matmul(
                              ps[:, 0:w],
                              qt[g][ho:ho + 64, 128 * r0i:128 * r0i + 128],
                              et[g][ho:ho + 64, m0 + c0:m0 + c0 + w],
                              start=True, stop=True)
                          if (c0 // 512) % 2 == 0:
                              nc.scalar.copy(p_sb[:, c0:c0 + w], ps[:, 0:w])
                          else:
                              nc.vector.tensor_copy(p_sb[:, c0:c0 + w], ps[:, 0:w])
                      nc.gpsimd.memset(p_sb[:, valid:valid + 1], 0.0)
                      nc.gpsimd.memset(p_sb[:, valid + 1:W], -1e9)
                      base = (h * NB + r0i) * 128 * WMAX
                      wr_ap = bass.AP(scratch, base, [[W, 128], [1, W]])
                      nc.sync.dma_start(wr_ap, p_sb[:, 0:W])

              # ---- attention ----
              # Scores for all 4 heads first: head pairs sit at lhsT
              # base_partition 0/64, so adjacent K=64 matmuls overlap in
              # separate PE row-groups. Then transposes+AV per head.
              for r0i in range(NB):
                  c_max = 128 * r0i + 128
                  W = c_max + 127
                  a_sbs = []
                  for h in range(HPC):
                      g, ho = h // 2, 64 * (h % 2)
                      base = (h * NB + r0i) * 128 * WMAX
                      s_sb = ss.tile([128, N], BF, tag="ssb", name="ssb")
                      rd_ap = bass.AP(scratch, base + 127, [[W - 1, 128], [1, c_max]])
                      nc.sync.dma_start(s_sb[:, 0:c_max], rd_ap)
                      a_sb = aa.tile([128, N], BF, tag="asb", name="asb")
                      a_sbs.append(a_sb)
                      for c0 in range(0, c_max, 512):
                          w = min(512, c_max - c0)
                          ps = psA.tile([128, 512], F32, tag="mm", name="mm")
                          nc.tensor.matmul(
                              ps[:, 0:w],
                              qt[g][ho:ho + 64, 1 + 128 * r0i:1 + 128 * r0i + 128],
                              kt[g][ho:ho + 64, c0:c0 + w],
                              start=True, stop=False)
                          nc.tensor.matmul(
                              ps[:, 0:w], ident[:], s_sb[:, c0:c0 + w],
                              start=False, stop=True)
                          nc.scalar.activation(
                              a_sb[:, c0:c0 + w], ps[:, 0:w],
                              mybir.ActivationFunctionType.Exp, scale=0.125)
                  for h in range(HPC):
                      g, ho = h // 2, 64 * (h % 2)
                      a_sb = a_sbs[h]
                      ps_o = psB.tile([128, 128], F32, tag="sm", name="sm")
                      for kb in range(r0i + 1):
                          ps_t = psB.tile([128, 128], BF, tag="sm", name="sm")
                          nc.tensor.transpose(ps_t[:], a_sb[:, bass.ts(kb, 128)], ident[:])
                          at_sb = atp.tile([128, 128], BF, tag="at", name="at")
                          nc.vector.tensor_copy(at_sb[:], ps_t[:])
                          nc.tensor.matmul(
                              ps_o[0:65, :], vaug[kb][:, h * 65:h * 65 + 65], at_sb[:],
                              start=(kb == 0), stop=(kb == r0i))
                      l_sb = llp.tile([1, 128], F32, tag="lsb", name="lsb")
                      nc.vector.tensor_copy(l_sb[:], ps_o[64:65, :])
                      linv = llp.tile([1, 128], F32, tag="linv", name="linv")
                      nc.vector.reciprocal(linv[:], l_sb[:])
                      ps_l = psB.tile([128, 128], F32, tag="sm", name="sm")
                      nc.tensor.matmul(ps_l[0:64, :], ones[:], linv[:],
                                       start=True, stop=True)
                      lb = llp.tile([64, 128], F32, tag="lb2", name="lb2")
                      nc.scalar.copy(lb[:], ps_l[0:64, :])
                      nc.vector.tensor_mul(
                          aot[g][ho:ho + 64, bass.ts(r0i, 128)],
                          ps_o[0:64, :], lb[:])

              # ---- output projection (partial over this core's head dims) ----
              for r0i in range(NB):
                  for nch in range(2):
                      ps = psA.tile([128, 512], F32, tag="mm", name="mm")
                      for dc in range(2):
                          nc.tensor.matmul(
                              ps[:], aot[dc][:, bass.ts(r0i, 128)],
                              wp[dc][:, bass.ts(nch, 512)],
                              start=(dc == 0), stop=(dc == 1))
                      o_sb = oo.tile([128, 512], F32, tag="osb", name="osb")
                      nc.vector.tensor_copy(o_sb[:], ps[:])
                      nc.sync.dma_start(
                          outp[bass.ts(r0i, 128), bass.ts(nch, 512)], o_sb[:])
    nc.compile()
    return nc


def kernel(x, Wq, Wk, Wv, Wp, bp, rel_embed):
    x = np.asarray(x, np.float32)
    bf = ml_dtypes.bfloat16
    if "nc" not in _CACHE:
        _CACHE["nc"] = _build_nc()
    nc = _CACHE["nc"]

    in_maps = []
    for c in range(8):
        b, hg = c // 4, c % 4
        c0 = hg * DC
        in_maps.append({
            "xT": np.ascontiguousarray(x[b].T).astype(bf),
            "wqT": np.ascontiguousarray(np.asarray(Wq)[c0:c0 + DC, :].T).astype(bf),
            "wkT": np.ascontiguousarray(np.asarray(Wk)[c0:c0 + DC, :].T).astype(bf),
            "wvT": np.ascontiguousarray(np.asarray(Wv)[c0:c0 + DC, :].T).astype(bf),
            "eT": np.ascontiguousarray(np.asarray(rel_embed)[:, c0:c0 + DC].T).astype(bf),
            "wpT": np.ascontiguousarray(np.asarray(Wp)[:, c0:c0 + DC].T).astype(bf),
        })
    kw = dict(_CACHE.get("run_kwargs") or {})
    r = run_bass_kernel_spmd(nc, in_maps, list(range(8)), **kw)
    _CACHE["last_result"] = r
    res = r.results
    out = np.zeros((2, N, D), np.float32)
    for c in range(8):
        out[c // 4] += np.asarray(res[c]["outp"], np.float32)
    out += np.asarray(bp, np.float32)
    return out



# revision 3
# speedup vs baseline: 1.4649x; 1.4649x over previous
"""Trainium2 Bass kernel for music-transformer relative attention — v3.

Shapes (hardcoded): x [2, 2048, 1024], 16 heads x 64 dims, MAXLEN == N == 2048.
Sharding: 8 cores = 2 batches x 4 head-groups (4 heads each). Each core computes
its heads' attention and a partial output projection (bf16); host sums the 4
partials per batch and adds the bias.

v3 structure: scores are computed TRANSPOSED in 512-wide query chunks:
S^T[j, i-chunk] = matmul(lhsT=k-block, rhs=q-chunk). The music-transformer
skew uses the v1 128-row DRAM bounce (contiguous write, (W-1)-strided read);
the srel tiles come back in NATURAL orientation and are added into the scores
PSUM with PE transpose-adds (matmul(ps_slice, lhsT=srel_tile, rhs=ident,
start=False)), so no extra transpose pass exists anywhere. exp output A^T
feeds the AV matmul directly. The diagonal-zero and causal -1e9 mask live at
fixed right-aligned columns of persistent p_sb staging tiles (preset once);
fully-masked j>i sub-tiles get a constant -1e9 tile added instead. P' bounce
blocks for chunk c+1 are emitted interleaved with attention chunk c so the
bounce DMA hides under attention matmuls; P'-PSUM evacuation copies run on
DVE/Pool only (ACT is reserved for exp).
"""

import sys

sys.path.insert(0, "/opt/trn_rl_repo")

import numpy as np
import ml_dtypes

import concourse.bass as bass
import concourse.tile as tile
from concourse import bacc
from concourse import mybir
from concourse.bass_utils import run_bass_kernel_spmd
from concourse.masks import make_identity

BF = mybir.dt.bfloat16
F32 = mybir.dt.float32
N = 2048
D = 1024
HD = 64
HPC = 4          # heads per core
DC = HPC * HD    # 256 head dims per core
CH = 512         # query chunk
NCH = N // CH    # 4 chunks
NB = N // 128    # 16 row blocks
PSB_W = 2175     # p_sb staging width = max W = 2048 + 127

_CACHE = {}


def _W(b):
    return 128 * (b + 1) + 127  # scratch row width for 128-row block b


_BASES = []
_off = 0
for _h in range(HPC):
    for _b in range(NB):
        _BASES.append(_off)
        _off += 128 * _W(_b)
SCRATCH_SZ = _off


def _base(h, b):
    return _BASES[h * NB + b]


def _build_nc():
    nc = bacc.Bacc()
    xT = nc.dram_tensor("xT", [D, N], BF, kind="ExternalInput")
    wqT = nc.dram_tensor("wqT", [D, DC], BF, kind="ExternalInput")
    wkT = nc.dram_tensor("wkT", [D, DC], BF, kind="ExternalInput")
    wvT = nc.dram_tensor("wvT", [D, DC], BF, kind="ExternalInput")
    eT = nc.dram_tensor("eT", [DC, N], BF, kind="ExternalInput")
    wpT = nc.dram_tensor("wpT", [DC, D], BF, kind="ExternalInput")
    outp = nc.dram_tensor("outp", [N, D], BF, kind="ExternalOutput")
    scratch = nc.dram_tensor("scratch", [SCRATCH_SZ], BF)

    from contextlib import ExitStack

    with tile.TileContext(nc) as tc, ExitStack() as ctx:
        pers = ctx.enter_context(tc.tile_pool(name="pers", bufs=1))
        psA = ctx.enter_context(tc.tile_pool(name="psA", bufs=4, space="PSUM"))
        psB = ctx.enter_context(tc.tile_pool(name="psB", bufs=2, space="PSUM"))
        psC = ctx.enter_context(tc.tile_pool(name="psC", bufs=2, space="PSUM"))
        ss = ctx.enter_context(tc.tile_pool(name="ss", bufs=2))
        aa = ctx.enter_context(tc.tile_pool(name="aa", bufs=4))
        oo = ctx.enter_context(tc.tile_pool(name="oo", bufs=4))
        llp = ctx.enter_context(tc.tile_pool(name="llp", bufs=3))

        # ---- persistent SBUF tensors ----
        xt = [pers.tile([128, N], BF, tag=f"xt{i}", name=f"xt{i}") for i in range(8)]
        wq = [pers.tile([128, DC], BF, tag=f"wq{i}", name=f"wq{i}") for i in range(8)]
        wk = [pers.tile([128, DC], BF, tag=f"wk{i}", name=f"wk{i}") for i in range(8)]
        wv = [pers.tile([128, DC], BF, tag=f"wv{i}", name=f"wv{i}") for i in range(8)]
        wp = [pers.tile([128, D], BF, tag=f"wp{i}", name=f"wp{i}") for i in range(2)]
        et = [pers.tile([128, N], BF, tag=f"et{i}", name=f"et{i}") for i in range(2)]
        qt = [pers.tile([128, N + 1], BF, tag=f"qt{i}", name=f"qt{i}") for i in range(2)]
        kt = [pers.tile([128, N], BF, tag=f"kt{i}", name=f"kt{i}") for i in range(2)]
        vaug = [pers.tile([128, HPC, HD + 1], BF, tag=f"va{i}", name=f"va{i}")
                for i in range(NB)]
        aot = [pers.tile([128, N], BF, tag=f"ao{i}", name=f"ao{i}") for i in range(2)]
        psb = [pers.tile([128, PSB_W], BF, tag=f"psb{i}", name=f"psb{i}")
               for i in range(8)]
        ident = pers.tile([128, 128], BF, tag="ident", name="ident")
        maskt = pers.tile([128, 128], BF, tag="maskt", name="maskt")
        ones = pers.tile([1, 64], F32, tag="ones", name="ones")

        make_identity(nc, ident[:])
        nc.gpsimd.memset(ones[:], 1.0)
        nc.gpsimd.memset(maskt[:], -1e9)
        for g in range(2):
            nc.gpsimd.memset(qt[g][:, 0:1], 0.0)
        # p_sb: data right-aligned so the diagonal-zero column sits at
        # PSB_W-128 and the 127 mask columns fill the tail — preset once.
        for i in range(len(psb)):
            nc.gpsimd.memset(psb[i][:, PSB_W - 128:PSB_W - 127], 0.0)
            nc.gpsimd.memset(psb[i][:, PSB_W - 127:PSB_W], -1e9)

        for i in range(8):
            nc.sync.dma_start(xt[i][:], xT[bass.ts(i, 128), :])
            nc.sync.dma_start(wq[i][:], wqT[bass.ts(i, 128), :])
            nc.sync.dma_start(wk[i][:], wkT[bass.ts(i, 128), :])
            nc.sync.dma_start(wv[i][:], wvT[bass.ts(i, 128), :])
        for g in range(2):
            nc.sync.dma_start(wp[g][:], wpT[bass.ts(g, 128), :])
            nc.sync.dma_start(et[g][:], eT[bass.ts(g, 128), :])

        # ---- P' bands (srel) -> skewed 128-row scratch blocks (v1 layout)
        # Block (h, b): row r holds P'[128b+r-1, m0+p]; data p in [0, valid),
        # diag zero at p=valid, mask -1e9 in (valid, W); valid = 128(b+1)-1,
        # W = valid + 128. Strided (W-1) re-read yields skewed srel rows.
        psb_state = {"idx": 0}

        def emit_pprime(h, c):
            g, ho = h // 2, 64 * (h % 2)
            for b in range(4 * c, 4 * c + 4):
                W = _W(b)
                valid = W - 128
                p_sb = psb[psb_state["idx"] % len(psb)]
                psb_state["idx"] += 1
                for c0 in range(0, valid, 512):
                    w = min(512, valid - c0)
                    ps = psA.tile([128, 512], F32, tag="mm", name="mm")
                    nc.tensor.matmul(
                        ps[:, 0:w],
                        qt[g][ho:ho + 64, 128 * b:128 * b + 128],
                        et[g][ho:ho + 64, N - valid + c0:N - valid + c0 + w],
                        start=True, stop=True)
                    dst = p_sb[:, PSB_W - W + c0:PSB_W - W + c0 + w]
                    # GPSIMD cannot read PSUM on HW: split DVE/ACT 2:1
                    if psb_state["idx"] % 3 == 2:
                        nc.scalar.copy(dst, ps[:, 0:w])
                    else:
                        nc.vector.tensor_copy(dst, ps[:, 0:w])
                wr_ap = bass.AP(scratch, _base(h, b), [[W, 128], [1, W]])
                nc.sync.dma_start(wr_ap, p_sb[:, PSB_W - W:PSB_W])

        # ---- projections: QT/KT transposed layout; the c=0 bounce blocks
        # are emitted right after each group's Q projection so the DRAM
        # round trip hides under the remaining projections.
        for g in range(2):
            for nchunk in range(4):
                ps = psA.tile([128, 512], F32, tag="mm", name="mm")
                for kc in range(8):
                    nc.tensor.matmul(
                        ps[:], wq[kc][:, bass.ts(g, 128)],
                        xt[kc][:, bass.ts(nchunk, 512)],
                        start=(kc == 0), stop=(kc == 7))
                nc.scalar.copy(qt[g][:, 1 + nchunk * 512:1 + (nchunk + 1) * 512], ps[:])
                ps2 = psA.tile([128, 512], F32, tag="mm", name="mm")
                for kc in range(8):
                    nc.tensor.matmul(
                        ps2[:], wk[kc][:, bass.ts(g, 128)],
                        xt[kc][:, bass.ts(nchunk, 512)],
                        start=(kc == 0), stop=(kc == 7))
                nc.vector.tensor_copy(kt[g][:, bass.ts(nchunk, 512)], ps2[:])
            emit_pprime(2 * g, 0)
            emit_pprime(2 * g + 1, 0)

        # ---- V natural layout + ones column ----
        for i in range(NB):
            ps = psA.tile([128, HPC, HD], F32, tag="mm", name="mm")
            for kc in range(8):
                nc.tensor.matmul(
                    ps[:, :, :], xt[kc][:, bass.ts(i, 128)], wv[kc][:],
                    start=(kc == 0), stop=(kc == 7))
            nc.gpsimd.memset(vaug[i][:, :, HD:HD + 1], 1.0)
            nc.vector.tensor_copy(vaug[i][:, :, 0:HD], ps[:, :, :])

        # Funnel cross-engine deps into PE's observed clock so no real
        # matmul needs >2 sync waits: dummy [1,1] matmuls reading each
        # phase-boundary tensor, cycling PSUM pools.
        srcs = [et[0], et[1], qt[0], qt[1], kt[0], kt[1], wp[0], wp[1]]
        for i, src in enumerate(srcs):
            if i % 2 == 0:
                ps_d = psA.tile([1, 1], F32, tag="mm", name="mm")
            else:
                ps_d = psB.tile([1, 1], F32, tag="sm", name="sm")
            nc.tensor.matmul(ps_d[0:1, 0:1], src[0:1, 1:2], src[0:1, 1:2],
                             start=True, stop=True)

        # ---- attention: transposed scores + srel transpose-add + exp + AV
        # The normalize tail (PE broadcast + DVE mul) of each unit is emitted
        # one unit later so its cross-engine latency never head-blocks the
        # PE queue.
        pending = []

        def flush_norm():
            while pending:
                fn = pending.pop(0)
                fn()

        def emit_attn(h, c):
            g, ho = h // 2, 64 * (h % 2)
            i0 = CH * c
            nkb = 4 * (c + 1)
            sn = []
            for t in range(4):
                b = 4 * c + t
                W = _W(b)
                cm = 128 * (b + 1)
                s_nat = ss.tile([128, cm], BF, tag=f"sn{t}", name=f"sn{t}")
                rd_ap = bass.AP(scratch, _base(h, b) + 127,
                                [[W - 1, 128], [1, cm]])
                nc.scalar.dma_start(s_nat[:], rd_ap)
                sn.append(s_nat)
            ps_o = psB.tile([65, CH], F32, tag="sm", name="sm")
            for kb in range(nkb):
                # Diagonal j-blocks only need scores for i >= 128*kb: shrink
                # the computed i-window; sub-diagonal (kb > 4c+t) tiles are
                # skipped entirely (their region is never written nor read).
                off = max(0, 128 * kb - i0)
                wdt = CH - off
                ps = psA.tile([128, CH], F32, tag="mm", name="mm")
                nc.tensor.matmul(
                    ps[:, off:CH],
                    kt[g][ho:ho + 64, bass.ts(kb, 128)],
                    qt[g][ho:ho + 64, 1 + i0 + off:1 + i0 + CH],
                    start=True, stop=False)
                t_lo = max(0, kb - 4 * c)
                for t in range(t_lo, 4):
                    nc.tensor.matmul(
                        ps[:, bass.ts(t, 128)], sn[t][:, bass.ts(kb, 128)],
                        ident[:], start=False, stop=(t == 3))
                a_sb = aa.tile([128, CH], BF, tag="asb", name="asb")
                nc.scalar.activation(
                    a_sb[:, off:CH], ps[:, off:CH],
                    mybir.ActivationFunctionType.Exp, scale=0.125)
                nc.tensor.matmul(
                    ps_o[:, off:CH], vaug[kb][:, h, :], a_sb[:, off:CH],
                    start=(kb == 0), stop=(kb == nkb - 1))
                if kb == 1:
                    flush_norm()
            # normalize: aot = ps_o[0:64] * (1 / l) broadcast over rows;
            # l_sb/linv now, broadcast+mul deferred.
            l_sb = llp.tile([1, CH], F32, tag="lsb", name="lsb")
            nc.vector.tensor_copy(l_sb[:], ps_o[64:65, :])
            linv = llp.tile([1, CH], F32, tag="linv", name="linv")
            nc.vector.reciprocal(linv[:], l_sb[:])

            def _norm(g=g, ho=ho, i0=i0, ps_o=ps_o, linv=linv):
                ps_l = psC.tile([64, CH], F32, tag="lb", name="lb")
                nc.tensor.matmul(ps_l[:], ones[:], linv[:], start=True,
                                 stop=True)
                lb = llp.tile([64, CH], F32, tag="lb2", name="lb2")
                nc.scalar.copy(lb[:], ps_l[:])
                nc.vector.tensor_mul(
                    aot[g][ho:ho + 64, i0:i0 + CH], ps_o[0:64, :], lb[:])
            pending.append(_norm)

        def emit_outproj(c):
            for r0i in range(4 * c, 4 * c + 4):
                for nch in range(2):
                    ps = psA.tile([128, 512], F32, tag="mm", name="mm")
                    for dc in range(2):
                        nc.tensor.matmul(
                            ps[:], aot[dc][:, bass.ts(r0i, 128)],
                            wp[dc][:, bass.ts(nch, 512)],
                            start=(dc == 0), stop=(dc == 1))
                    o_sb = oo.tile([128, 512], BF, tag="osb", name="osb")
                    nc.vector.tensor_copy(o_sb[:], ps[:])
                    nc.sync.dma_start(
                        outp[bass.ts(r0i, 128), bass.ts(nch, 512)], o_sb[:])

        # Interleave: bounce blocks for chunk c+1 alongside attention of
        # chunk c; the output projection for chunk c follows the first
        # attention unit of chunk c+1 (all heads' aot columns are complete
        # after flush).
        for c in range(NCH):
            for h in range(HPC):
                if c + 1 < NCH:
                    emit_pprime(h, c + 1)
                emit_attn(h, c)
            if c > 0:
                emit_outproj(c - 1)
        flush_norm()
        emit_outproj(NCH - 1)
    nc.compile()
    return nc


def kernel(x, Wq, Wk, Wv, Wp, bp, rel_embed):
    x = np.asarray(x, np.float32)
    bf = ml_dtypes.bfloat16
    if "nc" not in _CACHE:
        _CACHE["nc"] = _build_nc()
    nc = _CACHE["nc"]

    in_maps = []
    for c in range(8):
        b, hg = c // 4, c % 4
        c0 = hg * DC
        in_maps.append({
            "xT": np.ascontiguousarray(x[b].T).astype(bf),
            "wqT": np.ascontiguousarray(np.asarray(Wq)[c0:c0 + DC, :].T).astype(bf),
            "wkT": np.ascontiguousarray(np.asarray(Wk)[c0:c0 + DC, :].T).astype(bf),
            "wvT": np.ascontiguousarray(np.asarray(Wv)[c0:c0 + DC, :].T).astype(bf),
            "eT": np.ascontiguousarray(np.asarray(rel_embed)[:, c0:c0 + DC].T).astype(bf),
            "wpT": np.ascontiguousarray(np.asarray(Wp)[:, c0:c0 + DC].T).astype(bf),
        })
    kw = dict(_CACHE.get("run_kwargs") or {})
    r = run_bass_kernel_spmd(nc, in_maps, list(range(8)), **kw)
    _CACHE["last_result"] = r
    res = r.results
    out = np.zeros((2, N, D), np.float32)
    for c in range(8):
        out[c // 4] += np.asarray(res[c]["outp"], np.float32)
    out += np.asarray(bp, np.float32)
    return out


# revision 4
# speedup vs baseline: 1.5342x; 1.0473x over previous
"""Trainium2 Bass kernel for music-transformer relative attention — v3.

Shapes (hardcoded): x [2, 2048, 1024], 16 heads x 64 dims, MAXLEN == N == 2048.
Sharding: 8 cores = 2 batches x 4 head-groups (4 heads each). Each core computes
its heads' attention and a partial output projection (bf16); host sums the 4
partials per batch and adds the bias.

v3 structure: scores are computed TRANSPOSED in 512-wide query chunks:
S^T[j, i-chunk] = matmul(lhsT=k-block, rhs=q-chunk). The music-transformer
skew uses the v1 128-row DRAM bounce (contiguous write, (W-1)-strided read);
the srel tiles come back in NATURAL orientation and are added into the scores
PSUM with PE transpose-adds (matmul(ps_slice, lhsT=srel_tile, rhs=ident,
start=False)), so no extra transpose pass exists anywhere. exp output A^T
feeds the AV matmul directly. The diagonal-zero and causal -1e9 mask live at
fixed right-aligned columns of persistent p_sb staging tiles (preset once);
fully-masked j>i sub-tiles get a constant -1e9 tile added instead. P' bounce
blocks for chunk c+1 are emitted interleaved with attention chunk c so the
bounce DMA hides under attention matmuls; P'-PSUM evacuation copies run on
DVE/Pool only (ACT is reserved for exp).
"""

import sys

sys.path.insert(0, "/opt/trn_rl_repo")

import numpy as np
import ml_dtypes

import concourse.bass as bass
import concourse.tile as tile
from concourse import bacc
from concourse import mybir
from concourse.bass_utils import run_bass_kernel_spmd
from concourse.masks import make_identity

BF = mybir.dt.bfloat16
F32 = mybir.dt.float32
N = 2048
D = 1024
HD = 64
HPC = 4          # heads per core
DC = HPC * HD    # 256 head dims per core
CH = 512         # query chunk
NCH = N // CH    # 4 chunks
NB = N // 128    # 16 row blocks
PSB_W = 2175     # p_sb staging width = max W = 2048 + 127

_CACHE = {}


def _W(b):
    return 128 * (b + 1) + 127  # scratch row width for 128-row block b


_BASES = []
_off = 0
for _h in range(HPC):
    for _b in range(NB):
        _BASES.append(_off)
        _off += 128 * _W(_b)
SCRATCH_SZ = _off


def _base(h, b):
    return _BASES[h * NB + b]


def _build_nc():
    nc = bacc.Bacc()
    xT = nc.dram_tensor("xT", [D, N], BF, kind="ExternalInput")
    wqT = nc.dram_tensor("wqT", [D, DC], BF, kind="ExternalInput")
    wkT = nc.dram_tensor("wkT", [D, DC], BF, kind="ExternalInput")
    wvT = nc.dram_tensor("wvT", [D, DC], BF, kind="ExternalInput")
    eT = nc.dram_tensor("eT", [DC, N], BF, kind="ExternalInput")
    wpT = nc.dram_tensor("wpT", [DC, D], BF, kind="ExternalInput")
    outp = nc.dram_tensor("outp", [N, D], BF, kind="ExternalOutput")
    scratch = nc.dram_tensor("scratch", [SCRATCH_SZ], BF)

    from contextlib import ExitStack

    with tile.TileContext(nc) as tc, ExitStack() as ctx:
        pers = ctx.enter_context(tc.tile_pool(name="pers", bufs=1))
        psA = ctx.enter_context(tc.tile_pool(name="psA", bufs=3, space="PSUM"))
        psD = ctx.enter_context(tc.tile_pool(name="psD", bufs=2, space="PSUM"))
        psB = ctx.enter_context(tc.tile_pool(name="psB", bufs=2, space="PSUM"))
        psC = ctx.enter_context(tc.tile_pool(name="psC", bufs=1, space="PSUM"))
        ss = ctx.enter_context(tc.tile_pool(name="ss", bufs=3))
        aa = ctx.enter_context(tc.tile_pool(name="aa", bufs=4))
        oo = ctx.enter_context(tc.tile_pool(name="oo", bufs=4))
        llp = ctx.enter_context(tc.tile_pool(name="llp", bufs=4))

        # ---- persistent SBUF tensors ----
        xt = [pers.tile([128, N], BF, tag=f"xt{i}", name=f"xt{i}") for i in range(8)]
        wq = [pers.tile([128, DC], BF, tag=f"wq{i}", name=f"wq{i}") for i in range(8)]
        wk = [pers.tile([128, DC], BF, tag=f"wk{i}", name=f"wk{i}") for i in range(8)]
        wv = [pers.tile([128, DC], BF, tag=f"wv{i}", name=f"wv{i}") for i in range(8)]
        wp = [pers.tile([128, D], BF, tag=f"wp{i}", name=f"wp{i}") for i in range(2)]
        et = [pers.tile([128, N], BF, tag=f"et{i}", name=f"et{i}") for i in range(2)]
        qt = [pers.tile([128, N + 1], BF, tag=f"qt{i}", name=f"qt{i}") for i in range(2)]
        kt = [pers.tile([128, N], BF, tag=f"kt{i}", name=f"kt{i}") for i in range(2)]
        vaug = [pers.tile([128, HPC, HD + 1], BF, tag=f"va{i}", name=f"va{i}")
                for i in range(NB)]
        aot = [pers.tile([128, N], BF, tag=f"ao{i}", name=f"ao{i}") for i in range(2)]
        psb = [pers.tile([128, PSB_W], BF, tag=f"psb{i}", name=f"psb{i}")
               for i in range(8)]
        ident = pers.tile([128, 128], BF, tag="ident", name="ident")
        maskt = pers.tile([128, 128], BF, tag="maskt", name="maskt")
        ones = pers.tile([1, 64], F32, tag="ones", name="ones")

        make_identity(nc, ident[:])
        nc.gpsimd.memset(ones[:], 1.0)
        nc.gpsimd.memset(maskt[:], -1e9)
        for g in range(2):
            nc.gpsimd.memset(qt[g][:, 0:1], 0.0)
        # p_sb: data right-aligned so the diagonal-zero column sits at
        # PSB_W-128 and the 127 mask columns fill the tail — preset once.
        for i in range(len(psb)):
            nc.gpsimd.memset(psb[i][:, PSB_W - 128:PSB_W - 127], 0.0)
            nc.gpsimd.memset(psb[i][:, PSB_W - 127:PSB_W], -1e9)

        for i in range(8):
            nc.sync.dma_start(xt[i][:], xT[bass.ts(i, 128), :])
            nc.sync.dma_start(wq[i][:], wqT[bass.ts(i, 128), :])
            nc.sync.dma_start(wk[i][:], wkT[bass.ts(i, 128), :])
            nc.sync.dma_start(wv[i][:], wvT[bass.ts(i, 128), :])
        for g in range(2):
            nc.sync.dma_start(wp[g][:], wpT[bass.ts(g, 128), :])
            nc.sync.dma_start(et[g][:], eT[bass.ts(g, 128), :])

        # ---- P' bands (srel) -> skewed 128-row scratch blocks (v1 layout)
        # Block (h, b): row r holds P'[128b+r-1, m0+p]; data p in [0, valid),
        # diag zero at p=valid, mask -1e9 in (valid, W); valid = 128(b+1)-1,
        # W = valid + 128. Strided (W-1) re-read yields skewed srel rows.
        psb_state = {"idx": 0}

        def emit_pprime(h, c):
            g, ho = h // 2, 64 * (h % 2)
            for b in range(4 * c, 4 * c + 4):
                W = _W(b)
                valid = W - 128
                p_sb = psb[psb_state["idx"] % len(psb)]
                psb_state["idx"] += 1
                for c0 in range(0, valid, 512):
                    w = min(512, valid - c0)
                    ps = psD.tile([128, 512], F32, tag="pp", name="pp")
                    nc.tensor.matmul(
                        ps[:, 0:w],
                        qt[g][ho:ho + 64, 128 * b:128 * b + 128],
                        et[g][ho:ho + 64, N - valid + c0:N - valid + c0 + w],
                        start=True, stop=True)
                    dst = p_sb[:, PSB_W - W + c0:PSB_W - W + c0 + w]
                    # GPSIMD cannot read PSUM on HW: split DVE/ACT 2:1
                    if psb_state["idx"] % 3 == 2:
                        nc.scalar.copy(dst, ps[:, 0:w])
                    else:
                        nc.vector.tensor_copy(dst, ps[:, 0:w])
                wr_ap = bass.AP(scratch, _base(h, b), [[W, 128], [1, W]])
                nc.sync.dma_start(wr_ap, p_sb[:, PSB_W - W:PSB_W])

        # ---- projections: QT/KT transposed layout; the c=0 bounce blocks
        # are emitted right after each group's Q projection so the DRAM
        # round trip hides under the remaining projections.
        for g in range(2):
            for nchunk in range(4):
                ps = psA.tile([128, 512], F32, tag="mm", name="mm")
                for kc in range(8):
                    nc.tensor.matmul(
                        ps[:], wq[kc][:, bass.ts(g, 128)],
                        xt[kc][:, bass.ts(nchunk, 512)],
                        start=(kc == 0), stop=(kc == 7))
                nc.scalar.copy(qt[g][:, 1 + nchunk * 512:1 + (nchunk + 1) * 512], ps[:])
                ps2 = psA.tile([128, 512], F32, tag="mm", name="mm")
                for kc in range(8):
                    nc.tensor.matmul(
                        ps2[:], wk[kc][:, bass.ts(g, 128)],
                        xt[kc][:, bass.ts(nchunk, 512)],
                        start=(kc == 0), stop=(kc == 7))
                nc.vector.tensor_copy(kt[g][:, bass.ts(nchunk, 512)], ps2[:])
            emit_pprime(2 * g, 0)
            emit_pprime(2 * g + 1, 0)

        # ---- V natural layout + ones column ----
        for i in range(NB):
            ps = psA.tile([128, HPC, HD], F32, tag="mm", name="mm")
            for kc in range(8):
                nc.tensor.matmul(
                    ps[:, :, :], xt[kc][:, bass.ts(i, 128)], wv[kc][:],
                    start=(kc == 0), stop=(kc == 7))
            nc.gpsimd.memset(vaug[i][:, :, HD:HD + 1], 1.0)
            nc.vector.tensor_copy(vaug[i][:, :, 0:HD], ps[:, :, :])

        # Funnel cross-engine deps into PE's observed clock so no real
        # matmul needs >2 sync waits: dummy [1,1] matmuls reading each
        # phase-boundary tensor, cycling PSUM pools.
        srcs = [et[0], et[1], qt[0], qt[1], kt[0], kt[1], wp[0], wp[1]]
        for i, src in enumerate(srcs):
            if i % 2 == 0:
                ps_d = psA.tile([1, 1], F32, tag="mm", name="mm")
            else:
                ps_d = psB.tile([1, 1], F32, tag="sm", name="sm")
            nc.tensor.matmul(ps_d[0:1, 0:1], src[0:1, 1:2], src[0:1, 1:2],
                             start=True, stop=True)

        # ---- attention: transposed scores + srel transpose-add + exp + AV
        # The normalize tail (PE broadcast + DVE mul) of each unit is emitted
        # one unit later so its cross-engine latency never head-blocks the
        # PE queue.
        pending = []

        def flush_norm():
            while pending:
                fn = pending.pop(0)
                fn()

        def emit_attn(h, c):
            g, ho = h // 2, 64 * (h % 2)
            i0 = CH * c
            nkb = 4 * (c + 1)
            sn = []
            for t in range(4):
                b = 4 * c + t
                W = _W(b)
                cm = 128 * (b + 1)
                s_nat = ss.tile([128, cm], BF, tag=f"sn{t}", name=f"sn{t}")
                rd_ap = bass.AP(scratch, _base(h, b) + 127,
                                [[W - 1, 128], [1, cm]])
                nc.scalar.dma_start(s_nat[:], rd_ap)
                sn.append(s_nat)
            ps_o = psB.tile([65, CH], F32, tag="sm", name="sm")
            for kb in range(nkb):
                # Diagonal j-blocks only need scores for i >= 128*kb: shrink
                # the computed i-window; sub-diagonal (kb > 4c+t) tiles are
                # skipped entirely (their region is never written nor read).
                off = max(0, 128 * kb - i0)
                wdt = CH - off
                ps = psA.tile([128, CH], F32, tag="mm", name="mm")
                nc.tensor.matmul(
                    ps[:, off:CH],
                    kt[g][ho:ho + 64, bass.ts(kb, 128)],
                    qt[g][ho:ho + 64, 1 + i0 + off:1 + i0 + CH],
                    start=True, stop=False)
                t_lo = max(0, kb - 4 * c)
                for t in range(t_lo, 4):
                    nc.tensor.matmul(
                        ps[:, bass.ts(t, 128)], sn[t][:, bass.ts(kb, 128)],
                        ident[:], start=False, stop=(t == 3))
                a_sb = aa.tile([128, CH], BF, tag="asb", name="asb")
                nc.scalar.activation(
                    a_sb[:, off:CH], ps[:, off:CH],
                    mybir.ActivationFunctionType.Exp, scale=0.125)
                nc.tensor.matmul(
                    ps_o[:, off:CH], vaug[kb][:, h, :], a_sb[:, off:CH],
                    start=(kb == 0), stop=(kb == nkb - 1))
                if kb == 1:
                    flush_norm()
            # normalize: aot = ps_o[0:64] * (1 / l) broadcast over rows;
            # l_sb/linv now, broadcast+mul deferred.
            l_sb = llp.tile([1, CH], F32, tag="lsb", name="lsb")
            nc.vector.tensor_copy(l_sb[:], ps_o[64:65, :])
            linv = llp.tile([1, CH], F32, tag="linv", name="linv")
            nc.vector.reciprocal(linv[:], l_sb[:])

            def _norm(g=g, ho=ho, i0=i0, ps_o=ps_o, linv=linv):
                ps_l = psC.tile([64, CH], F32, tag="lb", name="lb")
                nc.tensor.matmul(ps_l[:], ones[:], linv[:], start=True,
                                 stop=True)
                lb = llp.tile([64, CH], F32, tag="lb2", name="lb2")
                nc.scalar.copy(lb[:], ps_l[:])
                nc.vector.tensor_mul(
                    aot[g][ho:ho + 64, i0:i0 + CH], ps_o[0:64, :], lb[:])
            pending.append(_norm)

        def emit_outproj(c):
            for r0i in range(4 * c, 4 * c + 4):
                for nch in range(2):
                    ps = psA.tile([128, 512], F32, tag="mm", name="mm")
                    for dc in range(2):
                        nc.tensor.matmul(
                            ps[:], aot[dc][:, bass.ts(r0i, 128)],
                            wp[dc][:, bass.ts(nch, 512)],
                            start=(dc == 0), stop=(dc == 1))
                    o_sb = oo.tile([128, 512], BF, tag="osb", name="osb")
                    nc.vector.tensor_copy(o_sb[:], ps[:])
                    nc.sync.dma_start(
                        outp[bass.ts(r0i, 128), bass.ts(nch, 512)], o_sb[:])

        # Interleave: bounce blocks for chunk c+1 alongside attention of
        # chunk c; the output projection for chunk c follows the first
        # attention unit of chunk c+1 (all heads' aot columns are complete
        # after flush).
        for c in range(NCH):
            for h in range(HPC):
                if c + 1 < NCH:
                    emit_pprime(h, c + 1)
                emit_attn(h, c)
            if c > 0:
                emit_outproj(c - 1)
        flush_norm()
        emit_outproj(NCH - 1)
    nc.compile()
    return nc


def kernel(x, Wq, Wk, Wv, Wp, bp, rel_embed):
    x = np.asarray(x, np.float32)
    bf = ml_dtypes.bfloat16
    if "nc" not in _CACHE:
        _CACHE["nc"] = _build_nc()
    nc = _CACHE["nc"]

    in_maps = []
    for c in range(8):
        b, hg = c // 4, c % 4
        c0 = hg * DC
        in_maps.append({
            "xT": np.ascontiguousarray(x[b].T).astype(bf),
            "wqT": np.ascontiguousarray(np.asarray(Wq)[c0:c0 + DC, :].T).astype(bf),
            "wkT": np.ascontiguousarray(np.asarray(Wk)[c0:c0 + DC, :].T).astype(bf),
            "wvT": np.ascontiguousarray(np.asarray(Wv)[c0:c0 + DC, :].T).astype(bf),
            "eT": np.ascontiguousarray(np.asarray(rel_embed)[:, c0:c0 + DC].T).astype(bf),
            "wpT": np.ascontiguousarray(np.asarray(Wp)[:, c0:c0 + DC].T).astype(bf),
        })
    kw = dict(_CACHE.get("run_kwargs") or {})
    r = run_bass_kernel_spmd(nc, in_maps, list(range(8)), **kw)
    _CACHE["last_result"] = r
    res = r.results
    out = np.zeros((2, N, D), np.float32)
    for c in range(8):
        out[c // 4] += np.asarray(res[c]["outp"], np.float32)
    out += np.asarray(bp, np.float32)
    return out


# revision 5
# speedup vs baseline: 1.5374x; 1.0021x over previous
"""Trainium2 Bass kernel for music-transformer relative attention — v3.

Shapes (hardcoded): x [2, 2048, 1024], 16 heads x 64 dims, MAXLEN == N == 2048.
Sharding: 8 cores = 2 batches x 4 head-groups (4 heads each). Each core computes
its heads' attention and a partial output projection (bf16); host sums the 4
partials per batch and adds the bias.

v3 structure: scores are computed TRANSPOSED in 512-wide query chunks:
S^T[j, i-chunk] = matmul(lhsT=k-block, rhs=q-chunk). The music-transformer
skew uses the v1 128-row DRAM bounce (contiguous write, (W-1)-strided read);
the srel tiles come back in NATURAL orientation and are added into the scores
PSUM with PE transpose-adds (matmul(ps_slice, lhsT=srel_tile, rhs=ident,
start=False)), so no extra transpose pass exists anywhere. exp output A^T
feeds the AV matmul directly. The diagonal-zero and causal -1e9 mask live at
fixed right-aligned columns of persistent p_sb staging tiles (preset once);
fully-masked j>i sub-tiles get a constant -1e9 tile added instead. P' bounce
blocks for chunk c+1 are emitted interleaved with attention chunk c so the
bounce DMA hides under attention matmuls; P'-PSUM evacuation copies run on
DVE/Pool only (ACT is reserved for exp).
"""

import sys

sys.path.insert(0, "/opt/trn_rl_repo")

import numpy as np
import ml_dtypes

import concourse.bass as bass
import concourse.tile as tile
from concourse import bacc
from concourse import mybir
from concourse.bass_utils import run_bass_kernel_spmd
from concourse.masks import make_identity

BF = mybir.dt.bfloat16
F32 = mybir.dt.float32
N = 2048
D = 1024
HD = 64
HPC = 4          # heads per core
DC = HPC * HD    # 256 head dims per core
CH = 512         # query chunk
NCH = N // CH    # 4 chunks
NB = N // 128    # 16 row blocks
PSB_W = 2175     # p_sb staging width = max W = 2048 + 127

_CACHE = {}


def _W(b):
    return 128 * (b + 1) + 127  # scratch row width for 128-row block b


_BASES = []
_off = 0
for _h in range(HPC):
    for _b in range(NB):
        _BASES.append(_off)
        _off += 128 * _W(_b)
SCRATCH_SZ = _off


def _base(h, b):
    return _BASES[h * NB + b]


def _build_nc():
    nc = bacc.Bacc()
    xT = nc.dram_tensor("xT", [D, N], BF, kind="ExternalInput")
    wqT = nc.dram_tensor("wqT", [D, DC], BF, kind="ExternalInput")
    wkT = nc.dram_tensor("wkT", [D, DC], BF, kind="ExternalInput")
    wvT = nc.dram_tensor("wvT", [D, DC], BF, kind="ExternalInput")
    eT = nc.dram_tensor("eT", [DC, N], BF, kind="ExternalInput")
    wpT = nc.dram_tensor("wpT", [DC, D], BF, kind="ExternalInput")
    outp = nc.dram_tensor("outp", [N, D], BF, kind="ExternalOutput")
    scratch = nc.dram_tensor("scratch", [SCRATCH_SZ], BF)

    from contextlib import ExitStack

    with tile.TileContext(nc) as tc, ExitStack() as ctx:
        pers = ctx.enter_context(tc.tile_pool(name="pers", bufs=1))
        psA = ctx.enter_context(tc.tile_pool(name="psA", bufs=3, space="PSUM"))
        psD = ctx.enter_context(tc.tile_pool(name="psD", bufs=2, space="PSUM"))
        psB = ctx.enter_context(tc.tile_pool(name="psB", bufs=2, space="PSUM"))
        psC = ctx.enter_context(tc.tile_pool(name="psC", bufs=1, space="PSUM"))
        ss = ctx.enter_context(tc.tile_pool(name="ss", bufs=4))
        aa = ctx.enter_context(tc.tile_pool(name="aa", bufs=4))
        oo = ctx.enter_context(tc.tile_pool(name="oo", bufs=4))
        llp = ctx.enter_context(tc.tile_pool(name="llp", bufs=2))

        # ---- persistent SBUF tensors ----
        xt = [pers.tile([128, N], BF, tag=f"xt{i}", name=f"xt{i}") for i in range(8)]
        wq = [pers.tile([128, DC], BF, tag=f"wq{i}", name=f"wq{i}") for i in range(8)]
        wk = [pers.tile([128, DC], BF, tag=f"wk{i}", name=f"wk{i}") for i in range(8)]
        wv = [pers.tile([128, DC], BF, tag=f"wv{i}", name=f"wv{i}") for i in range(8)]
        wp = [pers.tile([128, D], BF, tag=f"wp{i}", name=f"wp{i}") for i in range(2)]
        et = [pers.tile([128, N], BF, tag=f"et{i}", name=f"et{i}") for i in range(2)]
        qt = [pers.tile([128, N + 1], BF, tag=f"qt{i}", name=f"qt{i}") for i in range(2)]
        kt = [pers.tile([128, N], BF, tag=f"kt{i}", name=f"kt{i}") for i in range(2)]
        vaug = [pers.tile([128, HPC, HD + 1], BF, tag=f"va{i}", name=f"va{i}")
                for i in range(NB)]
        aot = [pers.tile([128, N], BF, tag=f"ao{i}", name=f"ao{i}") for i in range(2)]
        psb = [pers.tile([128, PSB_W], BF, tag=f"psb{i}", name=f"psb{i}")
               for i in range(8)]
        ident = pers.tile([128, 128], BF, tag="ident", name="ident")
        maskt = pers.tile([128, 128], BF, tag="maskt", name="maskt")
        ones = pers.tile([1, 64], F32, tag="ones", name="ones")

        make_identity(nc, ident[:])
        nc.gpsimd.memset(ones[:], 1.0)
        nc.gpsimd.memset(maskt[:], -1e9)
        for g in range(2):
            nc.gpsimd.memset(qt[g][:, 0:1], 0.0)
        # p_sb: data right-aligned so the diagonal-zero column sits at
        # PSB_W-128 and the 127 mask columns fill the tail — preset once.
        for i in range(len(psb)):
            nc.gpsimd.memset(psb[i][:, PSB_W - 128:PSB_W - 127], 0.0)
            nc.gpsimd.memset(psb[i][:, PSB_W - 127:PSB_W], -1e9)

        for i in range(8):
            nc.sync.dma_start(xt[i][:], xT[bass.ts(i, 128), :])
            nc.sync.dma_start(wq[i][:], wqT[bass.ts(i, 128), :])
            nc.sync.dma_start(wk[i][:], wkT[bass.ts(i, 128), :])
            nc.sync.dma_start(wv[i][:], wvT[bass.ts(i, 128), :])
        for g in range(2):
            nc.sync.dma_start(wp[g][:], wpT[bass.ts(g, 128), :])
            nc.sync.dma_start(et[g][:], eT[bass.ts(g, 128), :])

        # ---- P' bands (srel) -> skewed 128-row scratch blocks (v1 layout)
        # Block (h, b): row r holds P'[128b+r-1, m0+p]; data p in [0, valid),
        # diag zero at p=valid, mask -1e9 in (valid, W); valid = 128(b+1)-1,
        # W = valid + 128. Strided (W-1) re-read yields skewed srel rows.
        psb_state = {"idx": 0}

        def emit_pprime(h, c):
            g, ho = h // 2, 64 * (h % 2)
            for b in range(4 * c, 4 * c + 4):
                W = _W(b)
                valid = W - 128
                p_sb = psb[psb_state["idx"] % len(psb)]
                psb_state["idx"] += 1
                for c0 in range(0, valid, 512):
                    w = min(512, valid - c0)
                    ps = psD.tile([128, 512], F32, tag="pp", name="pp")
                    nc.tensor.matmul(
                        ps[:, 0:w],
                        qt[g][ho:ho + 64, 128 * b:128 * b + 128],
                        et[g][ho:ho + 64, N - valid + c0:N - valid + c0 + w],
                        start=True, stop=True)
                    dst = p_sb[:, PSB_W - W + c0:PSB_W - W + c0 + w]
                    # GPSIMD cannot read PSUM on HW: split DVE/ACT 2:1
                    if psb_state["idx"] % 3 == 2:
                        nc.scalar.copy(dst, ps[:, 0:w])
                    else:
                        nc.vector.tensor_copy(dst, ps[:, 0:w])
                wr_ap = bass.AP(scratch, _base(h, b), [[W, 128], [1, W]])
                nc.sync.dma_start(wr_ap, p_sb[:, PSB_W - W:PSB_W])

        # ---- projections: QT/KT transposed layout; the c=0 bounce blocks
        # are emitted right after each group's Q projection so the DRAM
        # round trip hides under the remaining projections.
        for g in range(2):
            for nchunk in range(4):
                ps = psA.tile([128, 512], F32, tag="mm", name="mm")
                for kc in range(8):
                    nc.tensor.matmul(
                        ps[:], wq[kc][:, bass.ts(g, 128)],
                        xt[kc][:, bass.ts(nchunk, 512)],
                        start=(kc == 0), stop=(kc == 7))
                nc.scalar.copy(qt[g][:, 1 + nchunk * 512:1 + (nchunk + 1) * 512], ps[:])
                ps2 = psA.tile([128, 512], F32, tag="mm", name="mm")
                for kc in range(8):
                    nc.tensor.matmul(
                        ps2[:], wk[kc][:, bass.ts(g, 128)],
                        xt[kc][:, bass.ts(nchunk, 512)],
                        start=(kc == 0), stop=(kc == 7))
                nc.vector.tensor_copy(kt[g][:, bass.ts(nchunk, 512)], ps2[:])
            emit_pprime(2 * g, 0)
            emit_pprime(2 * g + 1, 0)

        # ---- V natural layout + ones column ----
        for i in range(NB):
            ps = psA.tile([128, HPC, HD], F32, tag="mm", name="mm")
            for kc in range(8):
                nc.tensor.matmul(
                    ps[:, :, :], xt[kc][:, bass.ts(i, 128)], wv[kc][:],
                    start=(kc == 0), stop=(kc == 7))
            nc.gpsimd.memset(vaug[i][:, :, HD:HD + 1], 1.0)
            nc.vector.tensor_copy(vaug[i][:, :, 0:HD], ps[:, :, :])

        # Funnel cross-engine deps into PE's observed clock so no real
        # matmul needs >2 sync waits: dummy [1,1] matmuls reading each
        # phase-boundary tensor, cycling PSUM pools.
        srcs = [et[0], et[1], qt[0], qt[1], kt[0], kt[1], wp[0], wp[1]]
        for i, src in enumerate(srcs):
            if i % 2 == 0:
                ps_d = psA.tile([1, 1], F32, tag="mm", name="mm")
            else:
                ps_d = psB.tile([1, 1], F32, tag="sm", name="sm")
            nc.tensor.matmul(ps_d[0:1, 0:1], src[0:1, 1:2], src[0:1, 1:2],
                             start=True, stop=True)

        # ---- attention: transposed scores + srel transpose-add + exp + AV
        # The normalize tail (PE broadcast + DVE mul) of each unit is emitted
        # one unit later so its cross-engine latency never head-blocks the
        # PE queue.
        pending = []

        def flush_norm():
            while pending:
                fn = pending.pop(0)
                fn()

        def emit_attn(h, c):
            g, ho = h // 2, 64 * (h % 2)
            i0 = CH * c
            nkb = 4 * (c + 1)
            sn = []
            for t in range(4):
                b = 4 * c + t
                W = _W(b)
                cm = 128 * (b + 1)
                s_nat = ss.tile([128, cm], BF, tag=f"sn{t}", name=f"sn{t}")
                rd_ap = bass.AP(scratch, _base(h, b) + 127,
                                [[W - 1, 128], [1, cm]])
                nc.scalar.dma_start(s_nat[:], rd_ap)
                sn.append(s_nat)
            ps_o = psB.tile([65, CH], F32, tag="sm", name="sm")
            for kb in range(nkb):
                # Diagonal j-blocks only need scores for i >= 128*kb: shrink
                # the computed i-window; sub-diagonal (kb > 4c+t) tiles are
                # skipped entirely (their region is never written nor read).
                off = max(0, 128 * kb - i0)
                wdt = CH - off
                ps = psA.tile([128, CH], F32, tag="mm", name="mm")
                nc.tensor.matmul(
                    ps[:, off:CH],
                    kt[g][ho:ho + 64, bass.ts(kb, 128)],
                    qt[g][ho:ho + 64, 1 + i0 + off:1 + i0 + CH],
                    start=True, stop=False)
                t_lo = max(0, kb - 4 * c)
                for t in range(t_lo, 4):
                    nc.tensor.matmul(
                        ps[:, bass.ts(t, 128)], sn[t][:, bass.ts(kb, 128)],
                        ident[:], start=False, stop=(t == 3))
                a_sb = aa.tile([128, CH], BF, tag="asb", name="asb")
                nc.scalar.activation(
                    a_sb[:, off:CH], ps[:, off:CH],
                    mybir.ActivationFunctionType.Exp, scale=0.125)
                nc.tensor.matmul(
                    ps_o[:, off:CH], vaug[kb][:, h, :], a_sb[:, off:CH],
                    start=(kb == 0), stop=(kb == nkb - 1))
                if kb == 1:
                    flush_norm()
            # normalize: aot = ps_o[0:64] * (1 / l) broadcast over rows;
            # l_sb/linv now, broadcast+mul deferred.
            l_sb = llp.tile([1, CH], F32, tag="lsb", name="lsb")
            nc.vector.tensor_copy(l_sb[:], ps_o[64:65, :])
            linv = llp.tile([1, CH], F32, tag="linv", name="linv")
            nc.vector.reciprocal(linv[:], l_sb[:])

            def _norm(g=g, ho=ho, i0=i0, ps_o=ps_o, linv=linv):
                ps_l = psC.tile([64, CH], F32, tag="lb", name="lb")
                nc.tensor.matmul(ps_l[:], ones[:], linv[:], start=True,
                                 stop=True)
                lb = llp.tile([64, CH], F32, tag="lb2", name="lb2")
                nc.scalar.copy(lb[:], ps_l[:])
                nc.vector.tensor_mul(
                    aot[g][ho:ho + 64, i0:i0 + CH], ps_o[0:64, :], lb[:])
            pending.append(_norm)

        def emit_outproj(c):
            for r0i in range(4 * c, 4 * c + 4):
                for nch in range(2):
                    ps = psA.tile([128, 512], F32, tag="mm", name="mm")
                    for dc in range(2):
                        nc.tensor.matmul(
                            ps[:], aot[dc][:, bass.ts(r0i, 128)],
                            wp[dc][:, bass.ts(nch, 512)],
                            start=(dc == 0), stop=(dc == 1))
                    o_sb = oo.tile([128, 512], BF, tag="osb", name="osb")
                    nc.vector.tensor_copy(o_sb[:], ps[:])
                    nc.sync.dma_start(
                        outp[bass.ts(r0i, 128), bass.ts(nch, 512)], o_sb[:])

        # Interleave: bounce blocks for chunk c+1 alongside attention of
        # chunk c; the output projection for chunk c follows the first
        # attention unit of chunk c+1 (all heads' aot columns are complete
        # after flush).
        for c in range(NCH):
            for h in range(HPC):
                if c + 1 < NCH:
                    emit_pprime(h, c + 1)
                emit_attn(h, c)
            if c > 0:
                emit_outproj(c - 1)
        flush_norm()
        emit_outproj(NCH - 1)
    nc.compile()
    return nc


def kernel(x, Wq, Wk, Wv, Wp, bp, rel_embed):
    x = np.asarray(x, np.float32)
    bf = ml_dtypes.bfloat16
    if "nc" not in _CACHE:
        _CACHE["nc"] = _build_nc()
    nc = _CACHE["nc"]

    in_maps = []
    for c in range(8):
        b, hg = c // 4, c % 4
        c0 = hg * DC
        in_maps.append({
            "xT": np.ascontiguousarray(x[b].T).astype(bf),
            "wqT": np.ascontiguousarray(np.asarray(Wq)[c0:c0 + DC, :].T).astype(bf),
            "wkT": np.ascontiguousarray(np.asarray(Wk)[c0:c0 + DC, :].T).astype(bf),
            "wvT": np.ascontiguousarray(np.asarray(Wv)[c0:c0 + DC, :].T).astype(bf),
            "eT": np.ascontiguousarray(np.asarray(rel_embed)[:, c0:c0 + DC].T).astype(bf),
            "wpT": np.ascontiguousarray(np.asarray(Wp)[:, c0:c0 + DC].T).astype(bf),
        })
    kw = dict(_CACHE.get("run_kwargs") or {})
    r = run_bass_kernel_spmd(nc, in_maps, list(range(8)), **kw)
    _CACHE["last_result"] = r
    res = r.results
    out = np.zeros((2, N, D), np.float32)
    for c in range(8):
        out[c // 4] += np.asarray(res[c]["outp"], np.float32)
    out += np.asarray(bp, np.float32)
    return out


# revision 6
# speedup vs baseline: 1.5905x; 1.0345x over previous
"""Trainium2 Bass kernel for music-transformer relative attention — v3.

Shapes (hardcoded): x [2, 2048, 1024], 16 heads x 64 dims, MAXLEN == N == 2048.
Sharding: 8 cores = 2 batches x 4 head-groups (4 heads each). Each core computes
its heads' attention and a partial output projection (bf16); host sums the 4
partials per batch and adds the bias.

v3 structure: scores are computed TRANSPOSED in 512-wide query chunks:
S^T[j, i-chunk] = matmul(lhsT=k-block, rhs=q-chunk). The music-transformer
skew uses the v1 128-row DRAM bounce (contiguous write, (W-1)-strided read);
the srel tiles come back in NATURAL orientation and are added into the scores
PSUM with PE transpose-adds (matmul(ps_slice, lhsT=srel_tile, rhs=ident,
start=False)), so no extra transpose pass exists anywhere. exp output A^T
feeds the AV matmul directly. The diagonal-zero and causal -1e9 mask live at
fixed right-aligned columns of persistent p_sb staging tiles (preset once);
fully-masked j>i sub-tiles get a constant -1e9 tile added instead. P' bounce
blocks for chunk c+1 are emitted interleaved with attention chunk c so the
bounce DMA hides under attention matmuls; P'-PSUM evacuation copies run on
DVE/Pool only (ACT is reserved for exp).
"""

import sys

sys.path.insert(0, "/opt/trn_rl_repo")

import numpy as np
import ml_dtypes

import concourse.bass as bass
import concourse.tile as tile
from concourse import bacc
from concourse import mybir
from concourse.bass_utils import run_bass_kernel_spmd
from concourse.masks import make_identity

BF = mybir.dt.bfloat16
F8 = mybir.dt.float8e4
F32 = mybir.dt.float32
N = 2048
D = 1024
HD = 64
HPC = 4          # heads per core
DC = HPC * HD    # 256 head dims per core
CH = 512         # query chunk
NCH = N // CH    # 4 chunks
NB = N // 128    # 16 row blocks
PSB_W = 2175     # p_sb staging width = max W = 2048 + 127

_CACHE = {}


def _W(b):
    return 128 * (b + 1) + 127  # scratch row width for 128-row block b


_BASES = []
_off = 0
for _h in range(HPC):
    for _b in range(NB):
        _BASES.append(_off)
        _off += 128 * _W(_b)
SCRATCH_SZ = _off


def _base(h, b):
    return _BASES[h * NB + b]


def _build_nc():
    nc = bacc.Bacc()
    xT = nc.dram_tensor("xT", [D, N], BF, kind="ExternalInput")
    wqT = nc.dram_tensor("wqT", [D, DC], BF, kind="ExternalInput")
    wkT = nc.dram_tensor("wkT", [D, DC], BF, kind="ExternalInput")
    wvT = nc.dram_tensor("wvT", [D, DC], BF, kind="ExternalInput")
    eT = nc.dram_tensor("eT", [DC, N], BF, kind="ExternalInput")
    wpT = nc.dram_tensor("wpT", [DC, D], BF, kind="ExternalInput")
    outp = nc.dram_tensor("outp", [N, D], BF, kind="ExternalOutput")
    scratch = nc.dram_tensor("scratch", [SCRATCH_SZ], F8)

    from contextlib import ExitStack

    with tile.TileContext(nc) as tc, ExitStack() as ctx:
        pers = ctx.enter_context(tc.tile_pool(name="pers", bufs=1))
        psA = ctx.enter_context(tc.tile_pool(name="psA", bufs=3, space="PSUM"))
        psD = ctx.enter_context(tc.tile_pool(name="psD", bufs=2, space="PSUM"))
        psB = ctx.enter_context(tc.tile_pool(name="psB", bufs=2, space="PSUM"))
        psC = ctx.enter_context(tc.tile_pool(name="psC", bufs=1, space="PSUM"))
        ss = ctx.enter_context(tc.tile_pool(name="ss", bufs=4))
        aa = ctx.enter_context(tc.tile_pool(name="aa", bufs=4))
        oo = ctx.enter_context(tc.tile_pool(name="oo", bufs=4))
        llp = ctx.enter_context(tc.tile_pool(name="llp", bufs=2))

        # ---- persistent SBUF tensors ----
        xt = [pers.tile([128, N], BF, tag=f"xt{i}", name=f"xt{i}") for i in range(8)]
        wq = [pers.tile([128, DC], BF, tag=f"wq{i}", name=f"wq{i}") for i in range(8)]
        wk = [pers.tile([128, DC], BF, tag=f"wk{i}", name=f"wk{i}") for i in range(8)]
        wv = [pers.tile([128, DC], BF, tag=f"wv{i}", name=f"wv{i}") for i in range(8)]
        wp = [pers.tile([128, D], BF, tag=f"wp{i}", name=f"wp{i}") for i in range(2)]
        et = [pers.tile([128, N], BF, tag=f"et{i}", name=f"et{i}") for i in range(2)]
        qt = [pers.tile([128, N + 1], BF, tag=f"qt{i}", name=f"qt{i}") for i in range(2)]
        kt = [pers.tile([128, N], BF, tag=f"kt{i}", name=f"kt{i}") for i in range(2)]
        vaug = [pers.tile([128, HPC, HD + 1], BF, tag=f"va{i}", name=f"va{i}")
                for i in range(NB)]
        aot = [pers.tile([128, N], BF, tag=f"ao{i}", name=f"ao{i}") for i in range(2)]
        psb = [pers.tile([128, PSB_W], F8, tag=f"psb{i}", name=f"psb{i}")
               for i in range(8)]
        ident = pers.tile([128, 128], BF, tag="ident", name="ident")
        ident8 = pers.tile([128, 128], F8, tag="ident8", name="ident8")
        maskt = pers.tile([128, 128], BF, tag="maskt", name="maskt")
        ones = pers.tile([1, 64], F32, tag="ones", name="ones")

        make_identity(nc, ident[:])
        nc.vector.tensor_copy(ident8[:], ident[:])
        nc.gpsimd.memset(ones[:], 1.0)
        nc.gpsimd.memset(maskt[:], -1e9)
        for g in range(2):
            nc.gpsimd.memset(qt[g][:, 0:1], 0.0)
        # p_sb: data right-aligned so the diagonal-zero column sits at
        # PSB_W-128 and the 127 mask columns fill the tail — preset once.
        for i in range(len(psb)):
            nc.gpsimd.memset(psb[i][:, PSB_W - 128:PSB_W - 127], 0.0)
            nc.gpsimd.memset(psb[i][:, PSB_W - 127:PSB_W], -240.0)

        for i in range(8):
            nc.sync.dma_start(xt[i][:], xT[bass.ts(i, 128), :])
            nc.sync.dma_start(wq[i][:], wqT[bass.ts(i, 128), :])
            nc.sync.dma_start(wk[i][:], wkT[bass.ts(i, 128), :])
            nc.sync.dma_start(wv[i][:], wvT[bass.ts(i, 128), :])
        for g in range(2):
            nc.sync.dma_start(wp[g][:], wpT[bass.ts(g, 128), :])
            nc.sync.dma_start(et[g][:], eT[bass.ts(g, 128), :])

        # ---- P' bands (srel) -> skewed 128-row scratch blocks (v1 layout)
        # Block (h, b): row r holds P'[128b+r-1, m0+p]; data p in [0, valid),
        # diag zero at p=valid, mask -1e9 in (valid, W); valid = 128(b+1)-1,
        # W = valid + 128. Strided (W-1) re-read yields skewed srel rows.
        psb_state = {"idx": 0}

        def emit_pprime(h, c):
            g, ho = h // 2, 64 * (h % 2)
            for b in range(4 * c, 4 * c + 4):
                W = _W(b)
                valid = W - 128
                p_sb = psb[psb_state["idx"] % len(psb)]
                psb_state["idx"] += 1
                for c0 in range(0, valid, 512):
                    w = min(512, valid - c0)
                    ps = psD.tile([128, 512], F32, tag="pp", name="pp")
                    nc.tensor.matmul(
                        ps[:, 0:w],
                        qt[g][ho:ho + 64, 128 * b:128 * b + 128],
                        et[g][ho:ho + 64, N - valid + c0:N - valid + c0 + w],
                        start=True, stop=True)
                    dst = p_sb[:, PSB_W - W + c0:PSB_W - W + c0 + w]
                    # GPSIMD cannot read PSUM on HW: split DVE/ACT 2:1
                    if psb_state["idx"] % 3 == 2:
                        nc.scalar.copy(dst, ps[:, 0:w])
                    else:
                        nc.vector.tensor_copy(dst, ps[:, 0:w])
                wr_ap = bass.AP(scratch, _base(h, b), [[W, 128], [1, W]])
                nc.sync.dma_start(wr_ap, p_sb[:, PSB_W - W:PSB_W])

        # ---- projections: QT/KT transposed layout; the c=0 bounce blocks
        # are emitted right after each group's Q projection so the DRAM
        # round trip hides under the remaining projections.
        for g in range(2):
            for nchunk in range(4):
                ps = psA.tile([128, 512], F32, tag="mm", name="mm")
                for kc in range(8):
                    nc.tensor.matmul(
                        ps[:], wq[kc][:, bass.ts(g, 128)],
                        xt[kc][:, bass.ts(nchunk, 512)],
                        start=(kc == 0), stop=(kc == 7))
                nc.scalar.copy(qt[g][:, 1 + nchunk * 512:1 + (nchunk + 1) * 512], ps[:])
                ps2 = psA.tile([128, 512], F32, tag="mm", name="mm")
                for kc in range(8):
                    nc.tensor.matmul(
                        ps2[:], wk[kc][:, bass.ts(g, 128)],
                        xt[kc][:, bass.ts(nchunk, 512)],
                        start=(kc == 0), stop=(kc == 7))
                nc.vector.tensor_copy(kt[g][:, bass.ts(nchunk, 512)], ps2[:])
            emit_pprime(2 * g, 0)
            emit_pprime(2 * g + 1, 0)

        # ---- V natural layout + ones column ----
        for i in range(NB):
            ps = psA.tile([128, HPC, HD], F32, tag="mm", name="mm")
            for kc in range(8):
                nc.tensor.matmul(
                    ps[:, :, :], xt[kc][:, bass.ts(i, 128)], wv[kc][:],
                    start=(kc == 0), stop=(kc == 7))
            nc.gpsimd.memset(vaug[i][:, :, HD:HD + 1], 1.0)
            nc.vector.tensor_copy(vaug[i][:, :, 0:HD], ps[:, :, :])

        # Funnel cross-engine deps into PE's observed clock so no real
        # matmul needs >2 sync waits: dummy [1,1] matmuls reading each
        # phase-boundary tensor, cycling PSUM pools.
        srcs = [et[0], et[1], qt[0], qt[1], kt[0], kt[1], wp[0], wp[1]]
        for i, src in enumerate(srcs):
            if i % 2 == 0:
                ps_d = psA.tile([1, 1], F32, tag="mm", name="mm")
            else:
                ps_d = psB.tile([1, 1], F32, tag="sm", name="sm")
            nc.tensor.matmul(ps_d[0:1, 0:1], src[0:1, 1:2], src[0:1, 1:2],
                             start=True, stop=True)

        # ---- attention: transposed scores + srel transpose-add + exp + AV
        # The normalize tail (PE broadcast + DVE mul) of each unit is emitted
        # one unit later so its cross-engine latency never head-blocks the
        # PE queue.
        pending = []

        def flush_norm():
            while pending:
                fn = pending.pop(0)
                fn()

        def emit_attn(h, c):
            g, ho = h // 2, 64 * (h % 2)
            i0 = CH * c
            nkb = 4 * (c + 1)
            sn = []
            for t in range(4):
                b = 4 * c + t
                W = _W(b)
                cm = 128 * (b + 1)
                s_nat = ss.tile([128, cm], F8, tag=f"sn{t}", name=f"sn{t}")
                rd_ap = bass.AP(scratch, _base(h, b) + 127,
                                [[W - 1, 128], [1, cm]])
                nc.scalar.dma_start(s_nat[:], rd_ap)
                sn.append(s_nat)
            ps_o = psB.tile([65, CH], F32, tag="sm", name="sm")
            for kb in range(nkb):
                # Diagonal j-blocks only need scores for i >= 128*kb: shrink
                # the computed i-window; sub-diagonal (kb > 4c+t) tiles are
                # skipped entirely (their region is never written nor read).
                off = max(0, 128 * kb - i0)
                wdt = CH - off
                ps = psA.tile([128, CH], F32, tag="mm", name="mm")
                nc.tensor.matmul(
                    ps[:, off:CH],
                    kt[g][ho:ho + 64, bass.ts(kb, 128)],
                    qt[g][ho:ho + 64, 1 + i0 + off:1 + i0 + CH],
                    start=True, stop=False)
                t_lo = max(0, kb - 4 * c)
                for t in range(t_lo, 4):
                    nc.tensor.matmul(
                        ps[:, bass.ts(t, 128)], sn[t][:, bass.ts(kb, 128)],
                        ident8[:], start=False, stop=(t == 3))
                a_sb = aa.tile([128, CH], BF, tag="asb", name="asb")
                nc.scalar.activation(
                    a_sb[:, off:CH], ps[:, off:CH],
                    mybir.ActivationFunctionType.Exp, scale=0.125)
                nc.tensor.matmul(
                    ps_o[:, off:CH], vaug[kb][:, h, :], a_sb[:, off:CH],
                    start=(kb == 0), stop=(kb == nkb - 1))
                if kb == 1:
                    flush_norm()
            # normalize: aot = ps_o[0:64] * (1 / l) broadcast over rows;
            # l_sb/linv now, broadcast+mul deferred.
            l_sb = llp.tile([1, CH], F32, tag="lsb", name="lsb")
            nc.vector.tensor_copy(l_sb[:], ps_o[64:65, :])
            linv = llp.tile([1, CH], F32, tag="linv", name="linv")
            nc.vector.reciprocal(linv[:], l_sb[:])

            def _norm(g=g, ho=ho, i0=i0, ps_o=ps_o, linv=linv):
                ps_l = psC.tile([64, CH], F32, tag="lb", name="lb")
                nc.tensor.matmul(ps_l[:], ones[:], linv[:], start=True,
                                 stop=True)
                lb = llp.tile([64, CH], F32, tag="lb2", name="lb2")
                nc.scalar.copy(lb[:], ps_l[:])
                nc.vector.tensor_mul(
                    aot[g][ho:ho + 64, i0:i0 + CH], ps_o[0:64, :], lb[:])
            pending.append(_norm)

        def emit_outproj(c):
            for r0i in range(4 * c, 4 * c + 4):
                for nch in range(2):
                    ps = psA.tile([128, 512], F32, tag="mm", name="mm")
                    for dc in range(2):
                        nc.tensor.matmul(
                            ps[:], aot[dc][:, bass.ts(r0i, 128)],
                            wp[dc][:, bass.ts(nch, 512)],
                            start=(dc == 0), stop=(dc == 1))
                    o_sb = oo.tile([128, 512], BF, tag="osb", name="osb")
                    nc.vector.tensor_copy(o_sb[:], ps[:])
                    nc.sync.dma_start(
                        outp[bass.ts(r0i, 128), bass.ts(nch, 512)], o_sb[:])

        # Interleave: bounce blocks for chunk c+1 alongside attention of
        # chunk c; the output projection for chunk c follows the first
        # attention unit of chunk c+1 (all heads' aot columns are complete
        # after flush).
        for c in range(NCH):
            for h in range(HPC):
                if c + 1 < NCH:
                    emit_pprime(h, c + 1)
                emit_attn(h, c)
            if c > 0:
                emit_outproj(c - 1)
        flush_norm()
        emit_outproj(NCH - 1)
    nc.compile()
    return nc


def kernel(x, Wq, Wk, Wv, Wp, bp, rel_embed):
    x = np.asarray(x, np.float32)
    bf = ml_dtypes.bfloat16
    if "nc" not in _CACHE:
        _CACHE["nc"] = _build_nc()
    nc = _CACHE["nc"]

    in_maps = []
    for c in range(8):
        b, hg = c // 4, c % 4
        c0 = hg * DC
        in_maps.append({
            "xT": np.ascontiguousarray(x[b].T).astype(bf),
            "wqT": np.ascontiguousarray(np.asarray(Wq)[c0:c0 + DC, :].T).astype(bf),
            "wkT": np.ascontiguousarray(np.asarray(Wk)[c0:c0 + DC, :].T).astype(bf),
            "wvT": np.ascontiguousarray(np.asarray(Wv)[c0:c0 + DC, :].T).astype(bf),
            "eT": np.ascontiguousarray(np.asarray(rel_embed)[:, c0:c0 + DC].T).astype(bf),
            "wpT": np.ascontiguousarray(np.asarray(Wp)[:, c0:c0 + DC].T).astype(bf),
        })
    kw = dict(_CACHE.get("run_kwargs") or {})
    r = run_bass_kernel_spmd(nc, in_maps, list(range(8)), **kw)
    _CACHE["last_result"] = r
    res = r.results
    out = np.zeros((2, N, D), np.float32)
    for c in range(8):
        out[c // 4] += np.asarray(res[c]["outp"], np.float32)
    out += np.asarray(bp, np.float32)
    return out


# revision 7
# speedup vs baseline: 1.7034x; 1.0710x over previous
"""Trainium2 Bass kernel for music-transformer relative attention — v3.

Shapes (hardcoded): x [2, 2048, 1024], 16 heads x 64 dims, MAXLEN == N == 2048.
Sharding: 8 cores = 2 batches x 4 head-groups (4 heads each). Each core computes
its heads' attention and a partial output projection (bf16); host sums the 4
partials per batch and adds the bias.

v3 structure: scores are computed TRANSPOSED in 512-wide query chunks:
S^T[j, i-chunk] = matmul(lhsT=k-block, rhs=q-chunk). The music-transformer
skew uses the v1 128-row DRAM bounce (contiguous write, (W-1)-strided read);
the srel tiles come back in NATURAL orientation and are added into the scores
PSUM with PE transpose-adds (matmul(ps_slice, lhsT=srel_tile, rhs=ident,
start=False)), so no extra transpose pass exists anywhere. exp output A^T
feeds the AV matmul directly. The diagonal-zero and causal -1e9 mask live at
fixed right-aligned columns of persistent p_sb staging tiles (preset once);
fully-masked j>i sub-tiles get a constant -1e9 tile added instead. P' bounce
blocks for chunk c+1 are emitted interleaved with attention chunk c so the
bounce DMA hides under attention matmuls; P'-PSUM evacuation copies run on
DVE/Pool only (ACT is reserved for exp).
"""

import sys

sys.path.insert(0, "/opt/trn_rl_repo")

import numpy as np
import ml_dtypes

import concourse.bass as bass
import concourse.tile as tile
from concourse import bacc
from concourse import mybir
from concourse.bass_utils import run_bass_kernel_spmd
from concourse.masks import make_identity

BF = mybir.dt.bfloat16
F8 = mybir.dt.float8e4
F32 = mybir.dt.float32
N = 2048
D = 1024
HD = 64
HPC = 4          # heads per core
DC = HPC * HD    # 256 head dims per core
CH = 512         # query chunk
NCH = N // CH    # 4 chunks
NB = N // 128    # 16 row blocks
PSB_W = 2175     # p_sb staging width = max W = 2048 + 127

_CACHE = {}


def _W(b):
    return 128 * (b + 1) + 127  # scratch row width for 128-row block b


_BASES = []
_off = 0
for _h in range(HPC):
    for _b in range(NB):
        _BASES.append(_off)
        _off += 128 * _W(_b)
SCRATCH_SZ = _off


def _base(h, b):
    return _BASES[h * NB + b]


def _build_nc():
    nc = bacc.Bacc()
    xT = nc.dram_tensor("xT", [D, N], BF, kind="ExternalInput")
    wqT = nc.dram_tensor("wqT", [D, DC], BF, kind="ExternalInput")
    wkT = nc.dram_tensor("wkT", [D, DC], BF, kind="ExternalInput")
    wvT = nc.dram_tensor("wvT", [D, DC], BF, kind="ExternalInput")
    eT = nc.dram_tensor("eT", [DC, N], BF, kind="ExternalInput")
    wpT = nc.dram_tensor("wpT", [DC, D], BF, kind="ExternalInput")
    outp = nc.dram_tensor("outp", [N, D], BF, kind="ExternalOutput")
    scratch = nc.dram_tensor("scratch", [SCRATCH_SZ], F8)

    from contextlib import ExitStack

    with tile.TileContext(nc) as tc, ExitStack() as ctx:
        pers = ctx.enter_context(tc.tile_pool(name="pers", bufs=1))
        psA = ctx.enter_context(tc.tile_pool(name="psA", bufs=3, space="PSUM"))
        psD = ctx.enter_context(tc.tile_pool(name="psD", bufs=2, space="PSUM"))
        psB = ctx.enter_context(tc.tile_pool(name="psB", bufs=2, space="PSUM"))
        psC = ctx.enter_context(tc.tile_pool(name="psC", bufs=1, space="PSUM"))
        ss = ctx.enter_context(tc.tile_pool(name="ss", bufs=4))
        aa = ctx.enter_context(tc.tile_pool(name="aa", bufs=4))
        oo = ctx.enter_context(tc.tile_pool(name="oo", bufs=4))
        llp = ctx.enter_context(tc.tile_pool(name="llp", bufs=2))

        # ---- persistent SBUF tensors ----
        xt = [pers.tile([128, N], BF, tag=f"xt{i}", name=f"xt{i}") for i in range(8)]
        wq = [pers.tile([128, DC], BF, tag=f"wq{i}", name=f"wq{i}") for i in range(8)]
        wk = [pers.tile([128, DC], BF, tag=f"wk{i}", name=f"wk{i}") for i in range(8)]
        wv = [pers.tile([128, DC], BF, tag=f"wv{i}", name=f"wv{i}") for i in range(8)]
        wp = [pers.tile([128, D], BF, tag=f"wp{i}", name=f"wp{i}") for i in range(2)]
        et = [pers.tile([128, N], BF, tag=f"et{i}", name=f"et{i}") for i in range(2)]
        qt = [pers.tile([128, N + 1], BF, tag=f"qt{i}", name=f"qt{i}") for i in range(2)]
        kt = [pers.tile([128, N], BF, tag=f"kt{i}", name=f"kt{i}") for i in range(2)]
        vaug = [pers.tile([128, HPC, HD + 1], BF, tag=f"va{i}", name=f"va{i}")
                for i in range(NB)]
        aot = [pers.tile([128, N], BF, tag=f"ao{i}", name=f"ao{i}") for i in range(2)]
        psb = [pers.tile([128, PSB_W], F8, tag=f"psb{i}", name=f"psb{i}")
               for i in range(8)]
        ident = pers.tile([128, 128], BF, tag="ident", name="ident")
        ident8 = pers.tile([128, 128], F8, tag="ident8", name="ident8")
        maskt = pers.tile([128, 128], BF, tag="maskt", name="maskt")
        ones = pers.tile([1, 64], F32, tag="ones", name="ones")

        make_identity(nc, ident[:])
        nc.vector.tensor_copy(ident8[:], ident[:])
        nc.gpsimd.memset(ones[:], 1.0)
        nc.gpsimd.memset(maskt[:], -1e9)
        for g in range(2):
            nc.gpsimd.memset(qt[g][:, 0:1], 0.0)
        # p_sb: data right-aligned so the diagonal-zero column sits at
        # PSB_W-128 and the 127 mask columns fill the tail — preset once.
        for i in range(len(psb)):
            nc.gpsimd.memset(psb[i][:, PSB_W - 128:PSB_W - 127], 0.0)
            nc.gpsimd.memset(psb[i][:, PSB_W - 127:PSB_W], -240.0)

        for i in range(8):
            nc.sync.dma_start(xt[i][:], xT[bass.ts(i, 128), :])
            nc.sync.dma_start(wq[i][:], wqT[bass.ts(i, 128), :])
            nc.sync.dma_start(wk[i][:], wkT[bass.ts(i, 128), :])
            nc.sync.dma_start(wv[i][:], wvT[bass.ts(i, 128), :])
        for g in range(2):
            nc.sync.dma_start(wp[g][:], wpT[bass.ts(g, 128), :])
            nc.sync.dma_start(et[g][:], eT[bass.ts(g, 128), :])

        # ---- P' bands (srel) -> skewed 128-row scratch blocks (v1 layout)
        # Block (h, b): row r holds P'[128b+r-1, m0+p]; data p in [0, valid),
        # diag zero at p=valid, mask -1e9 in (valid, W); valid = 128(b+1)-1,
        # W = valid + 128. Strided (W-1) re-read yields skewed srel rows.
        psb_state = {"idx": 0}

        def emit_pprime(h, c):
            g, ho = h // 2, 64 * (h % 2)
            for b in range(4 * c, 4 * c + 4):
                W = _W(b)
                valid = W - 128
                p_sb = psb[psb_state["idx"] % len(psb)]
                psb_state["idx"] += 1
                for c0 in range(0, valid, 512):
                    w = min(512, valid - c0)
                    ps = psD.tile([128, 512], F32, tag="pp", name="pp")
                    nc.tensor.matmul(
                        ps[:, 0:w],
                        qt[g][ho:ho + 64, 128 * b:128 * b + 128],
                        et[g][ho:ho + 64, N - valid + c0:N - valid + c0 + w],
                        start=True, stop=True)
                    dst = p_sb[:, PSB_W - W + c0:PSB_W - W + c0 + w]
                    # GPSIMD cannot read PSUM on HW: split DVE/ACT 2:1
                    if psb_state["idx"] % 3 == 2:
                        nc.scalar.copy(dst, ps[:, 0:w])
                    else:
                        nc.vector.tensor_copy(dst, ps[:, 0:w])
                wr_ap = bass.AP(scratch, _base(h, b), [[W, 128], [1, W]])
                nc.sync.dma_start(wr_ap, p_sb[:, PSB_W - W:PSB_W])

        # ---- projections: QT/KT transposed layout; the c=0 bounce blocks
        # are emitted right after each group's Q projection so the DRAM
        # round trip hides under the remaining projections.
        for g in range(2):
            for nchunk in range(4):
                ps = psA.tile([128, 512], F32, tag="mm", name="mm")
                for kc in range(8):
                    nc.tensor.matmul(
                        ps[:], wq[kc][:, bass.ts(g, 128)],
                        xt[kc][:, bass.ts(nchunk, 512)],
                        start=(kc == 0), stop=(kc == 7))
                nc.scalar.copy(qt[g][:, 1 + nchunk * 512:1 + (nchunk + 1) * 512], ps[:])
                ps2 = psA.tile([128, 512], F32, tag="mm", name="mm")
                for kc in range(8):
                    nc.tensor.matmul(
                        ps2[:], wk[kc][:, bass.ts(g, 128)],
                        xt[kc][:, bass.ts(nchunk, 512)],
                        start=(kc == 0), stop=(kc == 7))
                nc.vector.tensor_copy(kt[g][:, bass.ts(nchunk, 512)], ps2[:])
            emit_pprime(2 * g, 0)
            emit_pprime(2 * g + 1, 0)

        # ---- V natural layout + ones column ----
        for i in range(NB):
            ps = psA.tile([128, HPC, HD], F32, tag="mm", name="mm")
            for kc in range(8):
                nc.tensor.matmul(
                    ps[:, :, :], xt[kc][:, bass.ts(i, 128)], wv[kc][:],
                    start=(kc == 0), stop=(kc == 7))
            nc.gpsimd.memset(vaug[i][:, :, HD:HD + 1], 1.0)
            nc.vector.tensor_copy(vaug[i][:, :, 0:HD], ps[:, :, :])

        # Funnel cross-engine deps into PE's observed clock so no real
        # matmul needs >2 sync waits: dummy [1,1] matmuls reading each
        # phase-boundary tensor, cycling PSUM pools.
        srcs = [et[0], et[1], qt[0], qt[1], kt[0], kt[1], wp[0], wp[1]]
        for i, src in enumerate(srcs):
            if i % 2 == 0:
                ps_d = psA.tile([1, 1], F32, tag="mm", name="mm")
            else:
                ps_d = psB.tile([1, 1], F32, tag="sm", name="sm")
            nc.tensor.matmul(ps_d[0:1, 0:1], src[0:1, 1:2], src[0:1, 1:2],
                             start=True, stop=True)

        # ---- attention: transposed scores + srel transpose-add + exp + AV
        # The normalize tail (PE broadcast + DVE mul) of each unit is emitted
        # one unit later so its cross-engine latency never head-blocks the
        # PE queue.
        pending = []

        def flush_norm():
            while pending:
                fn = pending.pop(0)
                fn()

        def emit_attn(h, c):
            g, ho = h // 2, 64 * (h % 2)
            i0 = CH * c
            nkb = 4 * (c + 1)
            sn = []
            for t in range(4):
                b = 4 * c + t
                W = _W(b)
                cm = 128 * (b + 1)
                s_nat = ss.tile([128, cm], F8, tag=f"sn{t}", name=f"sn{t}")
                rd_ap = bass.AP(scratch, _base(h, b) + 127,
                                [[W - 1, 128], [1, cm]])
                nc.gpsimd.dma_start(s_nat[:], rd_ap)
                sn.append(s_nat)
            ps_o = psB.tile([65, CH], F32, tag="sm", name="sm")
            for kb in range(nkb):
                # Diagonal j-blocks only need scores for i >= 128*kb: shrink
                # the computed i-window; sub-diagonal (kb > 4c+t) tiles are
                # skipped entirely (their region is never written nor read).
                off = max(0, 128 * kb - i0)
                wdt = CH - off
                ps = psA.tile([128, CH], F32, tag="mm", name="mm")
                nc.tensor.matmul(
                    ps[:, off:CH],
                    kt[g][ho:ho + 64, bass.ts(kb, 128)],
                    qt[g][ho:ho + 64, 1 + i0 + off:1 + i0 + CH],
                    start=True, stop=False)
                t_lo = max(0, kb - 4 * c)
                for t in range(t_lo, 4):
                    nc.tensor.matmul(
                        ps[:, bass.ts(t, 128)], sn[t][:, bass.ts(kb, 128)],
                        ident8[:], start=False, stop=(t == 3))
                a_sb = aa.tile([128, CH], BF, tag="asb", name="asb")
                nc.scalar.activation(
                    a_sb[:, off:CH], ps[:, off:CH],
                    mybir.ActivationFunctionType.Exp, scale=0.125)
                nc.tensor.matmul(
                    ps_o[:, off:CH], vaug[kb][:, h, :], a_sb[:, off:CH],
                    start=(kb == 0), stop=(kb == nkb - 1))
                if kb == 1:
                    flush_norm()
            # normalize: aot = ps_o[0:64] * (1 / l) broadcast over rows;
            # l_sb/linv now, broadcast+mul deferred.
            l_sb = llp.tile([1, CH], F32, tag="lsb", name="lsb")
            nc.vector.tensor_copy(l_sb[:], ps_o[64:65, :])
            linv = llp.tile([1, CH], F32, tag="linv", name="linv")
            nc.vector.reciprocal(linv[:], l_sb[:])

            def _norm(g=g, ho=ho, i0=i0, ps_o=ps_o, linv=linv):
                ps_l = psC.tile([64, CH], F32, tag="lb", name="lb")
                nc.tensor.matmul(ps_l[:], ones[:], linv[:], start=True,
                                 stop=True)
                lb = llp.tile([64, CH], F32, tag="lb2", name="lb2")
                nc.scalar.copy(lb[:], ps_l[:])
                nc.vector.tensor_mul(
                    aot[g][ho:ho + 64, i0:i0 + CH], ps_o[0:64, :], lb[:])
            pending.append(_norm)

        def emit_outproj(c):
            for r0i in range(4 * c, 4 * c + 4):
                for nch in range(2):
                    ps = psA.tile([128, 512], F32, tag="mm", name="mm")
                    for dc in range(2):
                        nc.tensor.matmul(
                            ps[:], aot[dc][:, bass.ts(r0i, 128)],
                            wp[dc][:, bass.ts(nch, 512)],
                            start=(dc == 0), stop=(dc == 1))
                    o_sb = oo.tile([128, 512], BF, tag="osb", name="osb")
                    nc.vector.tensor_copy(o_sb[:], ps[:])
                    nc.sync.dma_start(
                        outp[bass.ts(r0i, 128), bass.ts(nch, 512)], o_sb[:])

        # Interleave: bounce blocks for chunk c+1 alongside attention of
        # chunk c; the output projection for chunk c follows the first
        # attention unit of chunk c+1 (all heads' aot columns are complete
        # after flush).
        for c in range(NCH):
            for h in range(HPC):
                if c + 1 < NCH:
                    emit_pprime(h, c + 1)
                emit_attn(h, c)
            if c > 0:
                emit_outproj(c - 1)
        flush_norm()
        emit_outproj(NCH - 1)
    nc.compile()
    return nc


def kernel(x, Wq, Wk, Wv, Wp, bp, rel_embed):
    x = np.asarray(x, np.float32)
    bf = ml_dtypes.bfloat16
    if "nc" not in _CACHE:
        _CACHE["nc"] = _build_nc()
    nc = _CACHE["nc"]

    in_maps = []
    for c in range(8):
        b, hg = c // 4, c % 4
        c0 = hg * DC
        in_maps.append({
            "xT": np.ascontiguousarray(x[b].T).astype(bf),
            "wqT": np.ascontiguousarray(np.asarray(Wq)[c0:c0 + DC, :].T).astype(bf),
            "wkT": np.ascontiguousarray(np.asarray(Wk)[c0:c0 + DC, :].T).astype(bf),
            "wvT": np.ascontiguousarray(np.asarray(Wv)[c0:c0 + DC, :].T).astype(bf),
            "eT": np.ascontiguousarray(np.asarray(rel_embed)[:, c0:c0 + DC].T).astype(bf),
            "wpT": np.ascontiguousarray(np.asarray(Wp)[:, c0:c0 + DC].T).astype(bf),
        })
    kw = dict(_CACHE.get("run_kwargs") or {})
    r = run_bass_kernel_spmd(nc, in_maps, list(range(8)), **kw)
    _CACHE["last_result"] = r
    res = r.results
    out = np.zeros((2, N, D), np.float32)
    for c in range(8):
        out[c // 4] += np.asarray(res[c]["outp"], np.float32)
    out += np.asarray(bp, np.float32)
    return out
